# revision 12
# baseline (speedup 1.0000x reference)
"""Emformer block (pre-LN MHA + FFN, post-LN) on 8 Trainium2 NeuronCores.

Decomposition (zero replicated FLOPs, host reshard between phases):
  Phase 1 (token-sharded, 512 rows/core): LN0 -> qT/kT (channel-major, fp8)
           and v (channel-major, bf16) projections with fp8 DoubleRow
           matmuls (2 contraction tiles per instruction at 0.5 cycles/row).
           LN gamma/beta folded into weights on host; q/k/v scaled by 32 so
           fp8e4m3 sees ~unit-variance data.
  Phase 2 (head-sharded, 4 (b,h) pairs/core): scores^T = k^T.T @ q^T as fp8
           DoubleRow (contraction 64 = 2 k-tiles of 32), exp(s/8192 - 2) on
           ACT straight out of PSUM into bf16, mask multiply on DVE, attn^T
           and the softmax denominator via a 0.5-augmented v column, PSUM
           DMA'd straight to DRAM (no transposes).
  Phase 3 (token-sharded): residual + LN1 -> FFN1/FFN2 in fp8 DoubleRow with
           hi+lo error-split operands (3-term products) so fp8 quantization
           error cancels to ~bf16 level. Whole phase runs at 64x scale and
           relies on layer-norm scale invariance (eps scaled by 64^2) so no
           unscaling ops are needed.
"""

import ml_dtypes
import numpy as np

import concourse.bass as bass
import concourse.mybir as mybir
import concourse.tile as tile
from concourse import bacc
from concourse.bass_utils import run_bass_kernel_spmd
from concourse.masks import make_identity

F32 = mybir.dt.float32
F32R = mybir.dt.float32r
BF16 = mybir.dt.bfloat16
FP8 = mybir.dt.float8e4
AF = mybir.ActivationFunctionType
OP = mybir.AluOpType
DR = mybir.MatmulPerfMode.DoubleRow

NPF8 = ml_dtypes.float8_e4m3
NPBF = ml_dtypes.bfloat16

B, T, D, H, FFN = 2, 2048, 1024, 16, 4096
DH = D // H
LN_EPS = 1e-3
NCORES = 8
NTOK = B * T              # 4096
TOK_PC = NTOK // NCORES   # 512 token rows per core (phases 1/3)
NT = TOK_PC // 128        # 4 token tiles per core
CB = D // 128             # 8 blocks over D
DP = CB // 2              # 4 DoubleRow contraction pairs over D
FB = FFN // 128           # 32 blocks over FFN dim
JP = FB // 2              # 16 DoubleRow contraction pairs over FFN
NPAIR = (B * H) // NCORES # 4 (batch, head) pairs per core (phase 2)
MB = T // 128             # 16 key blocks
NBQ = T // 512            # 4 query blocks of 512

SQ = 32.0                 # q/k/v scale so fp8 sees ~N(0,1)*32 data
SEXP = 0.125 / (SQ * SQ)  # fold 1/sqrt(DH) and the q,k scales into exp
S3 = 64.0                 # phase-3 residual-stream scale (LN invariant)
EPS3 = LN_EPS * S3 * S3


def _ln_stats(nc, pool, xt, eps, rows=128, d=D):
    """Return (negmu, rstd) [rows,1] f32 tiles for layer norm over free dim."""
    nsub = d // 512
    stats = pool.tile([128, nsub, 6], F32, name="ln_stats", tag="ln_stats")
    xg = xt.rearrange("p (s q) -> p s q", s=nsub)
    for s in range(nsub):
        nc.vector.bn_stats(out=stats[:rows, s, :], in_=xg[:rows, s, :])
    mv = pool.tile([128, 2], F32, name="ln_mv", tag="ln_mv")
    nc.vector.bn_aggr(out=mv[:rows], in_=stats[:rows])
    negmu = pool.tile([128, 1], F32, name="ln_negmu", tag="ln_negmu")
    nc.vector.tensor_scalar_mul(negmu[:rows], mv[:rows, 0:1], -1.0)
    std = pool.tile([128, 1], F32, name="ln_std", tag="ln_std")
    eps_t = pool.tile([128, 1], F32, name="ln_eps", tag="ln_eps")
    nc.vector.memset(eps_t, eps)
    nc.scalar.activation(out=std[:rows], in_=mv[:rows, 1:2], func=AF.Sqrt,
                         bias=eps_t[:rows], scale=1.0)
    rstd = pool.tile([128, 1], F32, name="ln_rstd", tag="ln_rstd")
    nc.vector.reciprocal(out=rstd[:rows], in_=std[:rows])
    return negmu, rstd


def build_phase1():
    nc = bacc.Bacc(None, target_bir_lowering=False)
    x_d = nc.dram_tensor("x", [TOK_PC, D], BF16, kind="ExternalInput")
    w_d = {}
    for nm in ("q", "k", "v"):
        for hl in ("h", "l"):
            w_d[nm, hl] = nc.dram_tensor(f"w{nm}{hl}", [128, DP, 2, D], FP8,
                                         kind="ExternalInput")
    bq_d = nc.dram_tensor("bq", [128, CB], F32, kind="ExternalInput")
    bk_d = nc.dram_tensor("bk", [128, CB], F32, kind="ExternalInput")
    bv_d = nc.dram_tensor("bv", [128, CB], F32, kind="ExternalInput")
    qT_o = nc.dram_tensor("qT", [D, TOK_PC], BF16, kind="ExternalOutput")
    kT_o = nc.dram_tensor("kT", [D, TOK_PC], BF16, kind="ExternalOutput")
    v_o = nc.dram_tensor("v", [D, TOK_PC], BF16, kind="ExternalOutput")

    with tile.TileContext(nc) as tc:
        with (
            tc.tile_pool(name="const", bufs=1) as const,
            tc.tile_pool(name="w", bufs=1) as wpool,
            tc.tile_pool(name="xin", bufs=2) as xin,
            tc.tile_pool(name="small", bufs=4) as small,
            tc.tile_pool(name="ln", bufs=2) as lnp,
            tc.tile_pool(name="lnT", bufs=1) as lnTp,
            tc.tile_pool(name="qout", bufs=4) as qout,
            tc.tile_pool(name="pst", bufs=2, space="PSUM") as pst,
            tc.tile_pool(name="psq", bufs=4, space="PSUM") as psq,
        ):
            ident = const.tile([128, 128], BF16)
            make_identity(nc, ident)
            b_sb = {}
            for name, bd in (("q", bq_d), ("k", bk_d), ("v", bv_d)):
                t = const.tile([128, CB], F32, name=f"b{name}")
                nc.sync.dma_start(out=t, in_=bd[:, :])
                b_sb[name] = t
            w_sb = {}
            for (name, hl), wd in w_d.items():
                t = wpool.tile([128, DP, 2, D], FP8, name=f"w{name}{hl}",
                               tag=f"w{name}{hl}")
                nc.sync.dma_start(out=t, in_=wd[:, :, :, :])
                w_sb[name, hl] = t

            # LN0 -> xn (bf16) -> PE transpose -> lnT fp8 [128, CB, TOK_PC]
            lnT = lnTp.tile([128, CB, TOK_PC], FP8, name="lnT", tag="lnT")
            for nt in range(NT):
                xt = xin.tile([128, D], BF16, name="xt", tag="xt")
                nc.sync.dma_start(out=xt,
                                  in_=x_d[nt * 128:(nt + 1) * 128, :])
                negmu, rstd = _ln_stats(nc, small, xt, LN_EPS)
                xn = lnp.tile([128, D], BF16, name="xn", tag="xn")
                nc.vector.tensor_scalar(out=xn, in0=xt, scalar1=negmu,
                                        scalar2=rstd, op0=OP.add, op1=OP.mult)
                for dg in range(2):
                    tp = pst.tile([128, 4, 128], BF16, name="tp", tag="tp")
                    for j in range(4):
                        cb = dg * 4 + j
                        nc.tensor.transpose(
                            tp[:, j, :], xn[:, cb * 128:(cb + 1) * 128],
                            ident)
                    nc.scalar.activation(
                        out=lnT[:, dg * 4:(dg + 1) * 4,
                                nt * 128:(nt + 1) * 128],
                        in_=tp, func=AF.Copy)

            # QKV projections: DoubleRow fp8, out channel-major [128, 512]
            for name, out_d in (("q", qT_o), ("k", kT_o), ("v", v_o)):
                for ob in range(CB):
                    ps = psq.tile([128, TOK_PC], F32, name="psq", tag="psq")
                    first = True
                    for hl in ("h", "l"):
                        for dp in range(DP):
                            nc.tensor.matmul(
                                ps,
                                w_sb[name, hl][:, dp, :,
                                               ob * 128:(ob + 1) * 128],
                                lnT[:, 2 * dp:2 * dp + 2, :],
                                start=first,
                                stop=(hl == "l" and dp == DP - 1),
                                perf_mode=DR)
                            first = False
                    ot = qout.tile([128, TOK_PC], BF16, name="qvt",
                                   tag="qkv_out")
                    if name == "v":
                        nc.vector.tensor_scalar(
                            out=ot, in0=ps, scalar1=b_sb[name][:, ob:ob + 1],
                            scalar2=None, op0=OP.add)
                    else:
                        nc.scalar.activation(
                            out=ot, in_=ps, func=AF.Identity,
                            bias=b_sb[name][:, ob:ob + 1], scale=1.0)
                    nc.sync.dma_start(out=out_d[ob * 128:(ob + 1) * 128, :],
                                      in_=ot)

    nc.compile()
    return nc


def build_phase2():
    nc = bacc.Bacc(None, target_bir_lowering=False)
    qT_d = nc.dram_tensor("qT", [NPAIR, DH, T], BF16, kind="ExternalInput")
    kT_d = nc.dram_tensor("kT", [NPAIR, DH, T], BF16, kind="ExternalInput")
    v_d = nc.dram_tensor("v", [NPAIR, T, DH + 1], BF16, kind="ExternalInput")
    mT_d = nc.dram_tensor("maskT", [B, T, T], BF16, kind="ExternalInput")
    attn_o = nc.dram_tensor("attn", [NPAIR, DH + 1, T], F32,
                            kind="ExternalOutput")

    # key-block grouping: (start, size, pool_id) over the 16 key blocks
    GROUPS = [(0, 4, 0), (4, 2, 1), (6, 4, 0), (10, 2, 1), (12, 4, 0)]

    with tile.TileContext(nc) as tc:
        with (
            tc.tile_pool(name="const", bufs=1) as const,
            tc.tile_pool(name="mask", bufs=2) as maskp,
            tc.tile_pool(name="qk", bufs=2) as qkp,
            tc.tile_pool(name="vp", bufs=2) as vp,
            tc.tile_pool(name="em", bufs=2) as emp,
            tc.tile_pool(name="at", bufs=3) as atp,
            tc.tile_pool(name="psA", bufs=1, space="PSUM") as psA,
            tc.tile_pool(name="psB", bufs=1, space="PSUM") as psB,
            tc.tile_pool(name="psa", bufs=2, space="PSUM") as psa,
        ):
            neg2 = const.tile([128, 1], F32)
            nc.vector.memset(neg2, -2.0)
            for b in range(B):
                qs, ks, vs = [], [], []
                for hp in range(2):
                    p = b * 2 + hp
                    qsb = qkp.tile([DH, T], BF16, name=f"qsb{hp}",
                                   tag=f"q{hp}")
                    ksb = qkp.tile([DH, T], BF16, name=f"ksb{hp}",
                                   tag=f"k{hp}")
                    nc.sync.dma_start(out=qsb, in_=qT_d[p])
                    nc.sync.dma_start(out=ksb, in_=kT_d[p])
                    vsb = vp.tile([128, MB, DH + 1], BF16, name=f"vsb{hp}",
                                  tag=f"v{hp}")
                    nc.sync.dma_start(
                        out=vsb,
                        in_=v_d[p].rearrange("(mb mp) d -> mp mb d", mp=128))
                    qs.append(qsb); ks.append(ksb); vs.append(vsb)
                for nb in range(NBQ):
                    ns = slice(nb * 512, (nb + 1) * 512)
                    mt = maskp.tile([128, MB, 512], BF16, name="mt",
                                    tag="mask")
                    nc.sync.dma_start(
                        out=mt,
                        in_=mT_d[b].rearrange("(mb mp) n -> mp mb n",
                                              mp=128)[:, :, ns])
                    for hp in range(2):
                        p = b * 2 + hp
                        qsb, ksb, vsb = qs[hp], ks[hp], vs[hp]
                        em = emp.tile([128, MB, 512], BF16, name="em",
                                      tag="em")
                        for start, size, pid in GROUPS:
                            pool = psA if pid == 0 else psB
                            ps = pool.tile([128, size, 512], F32,
                                           name=f"ps{pid}", tag=f"ps{pid}")
                            for j in range(size):
                                kb = start + j
                                nc.tensor.matmul(
                                    ps[:, j, :],
                                    ksb[:, kb * 128:(kb + 1) * 128],
                                    qsb[:, ns],
                                    start=True, stop=True)
                            sl = slice(start, start + size)
                            nc.scalar.activation(
                                out=em[:, sl, :], in_=ps, func=AF.Exp,
                                scale=SEXP, bias=neg2)
                            nc.vector.tensor_tensor(
                                out=em[:, sl, :], in0=em[:, sl, :],
                                in1=mt[:, sl, :], op=OP.mult)
                        pa = psa.tile([DH + 1, 512], F32, name="pa",
                                      tag="attn")
                        for mb in range(MB):
                            nc.tensor.matmul(pa, vsb[:, mb, :],
                                             em[:, mb, :],
                                             start=(mb == 0),
                                             stop=(mb == MB - 1))
                        aT = atp.tile([DH + 1, 512], F32, name="aT",
                                      tag="aT")
                        nc.vector.tensor_copy(out=aT, in_=pa)
                        nc.sync.dma_start(out=attn_o[p, :, ns], in_=aT)

    nc.compile()
    return nc


def build_phase3():
    nc = bacc.Bacc(None, target_bir_lowering=False)
    attn_d = nc.dram_tensor("attn", [TOK_PC, D], F32, kind="ExternalInput")
    den_d = nc.dram_tensor("den", [TOK_PC, H], F32, kind="ExternalInput")
    x_d = nc.dram_tensor("x64", [TOK_PC, D], F32, kind="ExternalInput")
    w1h_d = nc.dram_tensor("w1h", [FB, 128, DP, 2, 128], FP8,
                           kind="ExternalInput")
    w1l_d = nc.dram_tensor("w1l", [FB, 128, DP, 2, 128], FP8,
                           kind="ExternalInput")
    bf1_d = nc.dram_tensor("bf1", [128, FB], F32, kind="ExternalInput")
    w2h_d = nc.dram_tensor("w2h", [2, 128, JP, 2, 512], FP8,
                           kind="ExternalInput")
    w2l_d = nc.dram_tensor("w2l", [2, 128, JP, 2, 512], FP8,
                           kind="ExternalInput")
    bf2_d = nc.dram_tensor("bf2", [1, D], F32R, kind="ExternalInput")
    ones_d = nc.dram_tensor("ones", [1, 128], F32R, kind="ExternalInput")
    g2_d = nc.dram_tensor("g2", [1, D], F32, kind="ExternalInput")
    b2_d = nc.dram_tensor("b2", [1, D], F32, kind="ExternalInput")
    out_o = nc.dram_tensor("out", [TOK_PC, D], F32, kind="ExternalOutput")

    with tile.TileContext(nc) as tc:
        with (
            tc.tile_pool(name="const", bufs=1) as const,
            tc.tile_pool(name="xin", bufs=2) as xin,
            tc.tile_pool(name="small", bufs=4) as small,
            tc.tile_pool(name="ao", bufs=1) as aop,
            tc.tile_pool(name="ln", bufs=2) as lnp,
            tc.tile_pool(name="lnT", bufs=1) as lnTp,
            tc.tile_pool(name="w1", bufs=4) as w1p,
            tc.tile_pool(name="w2", bufs=1) as w2p,
            tc.tile_pool(name="t1b", bufs=3) as t1bp,
            tc.tile_pool(name="t1", bufs=1) as t1p,
            tc.tile_pool(name="y", bufs=2) as yp,
            tc.tile_pool(name="outp", bufs=2) as outp,
            tc.tile_pool(name="pst", bufs=2, space="PSUM") as pst,
            tc.tile_pool(name="psf", bufs=2, space="PSUM") as psf,
            tc.tile_pool(name="psy", bufs=2, space="PSUM") as psyp,
        ):
            ident = const.tile([128, 128], BF16)
            make_identity(nc, ident)
            ones_t = const.tile([1, 128], F32R)
            nc.sync.dma_start(out=ones_t, in_=ones_d[:, :])
            bf1_sb = const.tile([128, FB], F32)
            nc.sync.dma_start(out=bf1_sb, in_=bf1_d[:, :])
            bf2_sb = const.tile([1, D], F32R)
            nc.sync.dma_start(out=bf2_sb, in_=bf2_d[:, :])
            g2_sb = const.tile([128, D], F32)
            nc.sync.dma_start(out=g2_sb, in_=bass.AP(
                tensor=g2_d, offset=0, ap=[[0, 128], [1, D]]))
            b2_sb = const.tile([128, D], F32)
            nc.sync.dma_start(out=b2_sb, in_=bass.AP(
                tensor=b2_d, offset=0, ap=[[0, 128], [1, D]]))

            # residual 1 (at 64x scale) + LN1 -> transpose -> hi/lo fp8
            lnTb = lnTp.tile([128, CB, TOK_PC], BF16, name="lnTb", tag="lnTb")
            lnTh = lnTp.tile([128, CB, TOK_PC], FP8, name="lnTh", tag="lnTh")
            lnTl = lnTp.tile([128, CB, TOK_PC], FP8, name="lnTl", tag="lnTl")
            ao_t = []
            for nt in range(NT):
                at = xin.tile([128, D], F32, name="at", tag="attn_in")
                nc.sync.dma_start(out=at,
                                  in_=attn_d[nt * 128:(nt + 1) * 128, :])
                xt = xin.tile([128, D], F32, name="xt", tag="x_in")
                nc.sync.dma_start(out=xt,
                                  in_=x_d[nt * 128:(nt + 1) * 128, :])
                den = small.tile([128, H], F32, name="den", tag="den")
                nc.sync.dma_start(out=den,
                                  in_=den_d[nt * 128:(nt + 1) * 128, :])
                rec = small.tile([128, H], F32, name="recd", tag="recd")
                nc.vector.reciprocal(out=rec, in_=den)
                rec_bc = bass.AP(tensor=rec.tensor, offset=rec.offset,
                                 ap=[rec.ap[0], rec.ap[1], [0, DH]])
                atg = at.rearrange("p (h d) -> p h d", h=H)
                nc.vector.tensor_tensor(out=atg, in0=atg, in1=rec_bc,
                                        op=OP.mult)
                ao = aop.tile([128, D], F32, name=f"ao{nt}", tag=f"ao{nt}")
                nc.vector.tensor_tensor(out=ao, in0=at, in1=xt, op=OP.add)
                ao_t.append(ao)
                negmu, rstd = _ln_stats(nc, small, ao, EPS3)
                ln_t = lnp.tile([128, D], BF16, name="ln1", tag="ln1")
                nc.vector.tensor_scalar(out=ln_t, in0=ao, scalar1=negmu,
                                        scalar2=rstd, op0=OP.add,
                                        op1=OP.mult)
                for dg in range(2):
                    tp = pst.tile([128, 4, 128], BF16, name="tp", tag="tp")
                    for j in range(4):
                        cb = dg * 4 + j
                        nc.tensor.transpose(
                            tp[:, j, :], ln_t[:, cb * 128:(cb + 1) * 128],
                            ident)
                    nc.scalar.activation(
                        out=lnTb[:, dg * 4:(dg + 1) * 4,
                                 nt * 128:(nt + 1) * 128],
                        in_=tp, func=AF.Copy)
            nc.scalar.activation(out=lnTh, in_=lnTb, func=AF.Copy)
            nc.vector.tensor_tensor(out=lnTl, in0=lnTb, in1=lnTh,
                                    op=OP.subtract)

            # FFN1 (3-term hi/lo), relu fused on ACT, t1 split hi/lo
            t1h = t1p.tile([128, FB, TOK_PC], FP8, name="t1h", tag="t1h")
            t1l = t1p.tile([128, FB, TOK_PC], FP8, name="t1l", tag="t1l")
            for fb in range(FB):
                w1h = w1p.tile([128, DP, 2, 128], FP8, name="w1h", tag="w1h")
                nc.sync.dma_start(out=w1h, in_=w1h_d[fb])
                w1l = w1p.tile([128, DP, 2, 128], FP8, name="w1l", tag="w1l")
                nc.sync.dma_start(out=w1l, in_=w1l_d[fb])
                ps = psf.tile([128, TOK_PC], F32, name="psf", tag="psf")
                first = True
                for wt, lt in ((w1h, lnTh), (w1l, lnTh), (w1h, lnTl)):
                    for dp in range(DP):
                        nc.tensor.matmul(
                            ps, wt[:, dp, :, :],
                            lt[:, 2 * dp:2 * dp + 2, :],
                            start=first,
                            stop=(wt is w1h and lt is lnTl and dp == DP - 1),
                            perf_mode=DR)
                        first = False
                t1b = t1bp.tile([128, TOK_PC], BF16, name="t1b", tag="t1b")
                nc.scalar.activation(out=t1b, in_=ps, func=AF.Relu,
                                     bias=bf1_sb[:, fb:fb + 1],
                                     scale=1.0 / SQ)
                nc.scalar.activation(out=t1h[:, fb, :], in_=t1b, func=AF.Copy)
                nc.gpsimd.tensor_tensor(out=t1l[:, fb, :], in0=t1b,
                                        in1=t1h[:, fb, :], op=OP.subtract)

            # FFN2 (3-term hi/lo) + bias row + residual, then LN2
            w2_sb = {}
            for dh in range(2):
                for nm, wd in (("h", w2h_d), ("l", w2l_d)):
                    t = w2p.tile([128, JP, 2, 512], FP8, name=f"w2{nm}{dh}",
                                 tag=f"w2{nm}{dh}")
                    nc.sync.dma_start(out=t, in_=wd[dh])
                    w2_sb[nm, dh] = t
            for nt in range(NT):
                tsl = slice(nt * 128, (nt + 1) * 128)
                y_t = yp.tile([128, D], F32, name=f"y{nt}", tag="y")
                for dh in range(2):
                    dsl = slice(dh * 512, (dh + 1) * 512)
                    py = psyp.tile([128, 512], F32, name="psy", tag="psy")
                    first = True
                    for tt, wn in ((t1h, "h"), (t1h, "l"), (t1l, "h")):
                        w2 = w2_sb[wn, dh]
                        for jp in range(JP):
                            nc.tensor.matmul(
                                py, tt[:, 2 * jp:2 * jp + 2, tsl],
                                w2[:, jp, :, :],
                                start=first, stop=False, perf_mode=DR)
                            first = False
                    nc.tensor.matmul(py, ones_t, bf2_sb[0:1, dsl],
                                     start=False, stop=True)
                    nc.vector.tensor_tensor(out=y_t[:, dsl], in0=py,
                                            in1=ao_t[nt][:, dsl], op=OP.add)
                negmu, rstd = _ln_stats(nc, small, y_t, EPS3)
                z = lnp.tile([128, D], F32, name="z", tag="z")
                nc.vector.tensor_scalar(out=z, in0=y_t, scalar1=negmu,
                                        scalar2=rstd, op0=OP.add,
                                        op1=OP.mult)
                ot = outp.tile([128, D], F32, name="ot", tag="out")
                nc.gpsimd.tensor_tensor(out=ot, in0=z, in1=g2_sb, op=OP.mult)
                nc.gpsimd.tensor_tensor(out=ot, in0=ot, in1=b2_sb, op=OP.add)
                nc.sync.dma_start(out=out_o[tsl, :], in_=ot)

    nc.compile()
    return nc


_CACHE = {}


def _get(name, builder):
    if name not in _CACHE:
        _CACHE[name] = builder()
    return _CACHE[name]


def _pack_w_in(w):
    """[D, N] -> DoubleRow lhsT layout [128, DP, 2, N] (fp8)."""
    return np.ascontiguousarray(
        np.asarray(w, np.float32).reshape(DP, 2, 128, -1)
        .transpose(2, 0, 1, 3)).astype(NPF8)


def kernel(x, mask, Wq, bq, Wk, bk, Wv, bv, g_in, b_in, g1, b1,
           W_ff1, b_ff1, W_ff2, b_ff2, g2, b2):
    f = np.float32
    x = np.asarray(x, f)
    xf = x.reshape(NTOK, D)
    Wq_s = SQ * (np.asarray(g_in, f)[:, None] * np.asarray(Wq, f))
    Wk_s = SQ * (np.asarray(g_in, f)[:, None] * np.asarray(Wk, f))
    Wv_s = SQ * (np.asarray(g_in, f)[:, None] * np.asarray(Wv, f))
    bq_f = (SQ * (b_in @ Wq + bq)).astype(f).reshape(CB, 128).T.copy()
    bk_f = (SQ * (b_in @ Wk + bk)).astype(f).reshape(CB, 128).T.copy()
    bv_f = (SQ * (b_in @ Wv + bv)).astype(f).reshape(CB, 128).T.copy()
    maskT = np.ascontiguousarray(
        np.asarray(mask)[:, 0].transpose(0, 2, 1)).astype(NPBF)

    W1_s = SQ * (np.asarray(g1, f)[:, None] * np.asarray(W_ff1, f))
    W1_hi = W1_s.astype(NPF8)
    W1_lo = (W1_s - W1_hi.astype(f)).astype(NPF8)
    bf1_f = (b1 @ W_ff1 + b_ff1).astype(f).reshape(FB, 128).T.copy()
    W2_s = S3 * np.asarray(W_ff2, f)
    W2_hi = W2_s.astype(NPF8)
    W2_lo = (W2_s - W2_hi.astype(f)).astype(NPF8)

    def pack_w1(w8):
        # [D, FFN] fp8 -> [FB, 128, DP, 2, 128]
        return np.ascontiguousarray(
            w8.reshape(DP, 2, 128, FB, 128).transpose(3, 2, 0, 1, 4))

    def pack_w2(w8):
        # [FFN, D] fp8 -> [2, 128, JP, 2, 512]
        return np.ascontiguousarray(
            w8.reshape(JP, 2, 128, 2, 512).transpose(3, 2, 0, 1, 4))

    ones = np.ones((1, 128), f)
    cores = list(range(NCORES))

    # ---- phase 1
    nc1 = _get("p1", build_phase1)
    w_in = {}
    for nm, Ws in (("q", Wq_s), ("k", Wk_s), ("v", Wv_s)):
        hi = Ws.astype(NPF8)
        lo = (Ws - hi.astype(f)).astype(NPF8)
        w_in[f"w{nm}h"] = _pack_w_in(hi)
        w_in[f"w{nm}l"] = _pack_w_in(lo)
    in1 = [{
        "x": xf[c * TOK_PC:(c + 1) * TOK_PC].astype(NPBF),
        "bq": bq_f, "bk": bk_f, "bv": bv_f, **w_in,
    } for c in cores]
    r1 = run_bass_kernel_spmd(nc1, in1, cores)
    qT = np.concatenate([r1.results[c]["qT"] for c in cores], axis=1)
    kT = np.concatenate([r1.results[c]["kT"] for c in cores], axis=1)
    vT = np.concatenate([r1.results[c]["v"] for c in cores], axis=1)

    # ---- phase 2
    nc2 = _get("p2", build_phase2)
    in2 = []
    for c in cores:
        qs, ks, vs = [], [], []
        for b in range(B):
            for hp in range(2):
                h = 2 * c + hp
                dsl = slice(h * DH, (h + 1) * DH)
                tsl = slice(b * T, (b + 1) * T)
                qs.append(qT[dsl, tsl])
                ks.append(kT[dsl, tsl])
                va = np.empty((T, DH + 1), NPBF)
                va[:, 0:DH] = vT[dsl, tsl].T
                va[:, DH] = NPBF(0.5)
                vs.append(va)
        in2.append({
            "qT": np.ascontiguousarray(np.stack(qs)),
            "kT": np.ascontiguousarray(np.stack(ks)),
            "v": np.ascontiguousarray(np.stack(vs)),
            "maskT": maskT,
        })
    r2 = run_bass_kernel_spmd(nc2, in2, cores)
    attn = np.empty((NTOK, D), f)
    den = np.empty((NTOK, H), f)
    for c in cores:
        i = 0
        for b in range(B):
            for hp in range(2):
                h = 2 * c + hp
                a65 = r2.results[c]["attn"][i]
                attn[b * T:(b + 1) * T, h * DH:(h + 1) * DH] = a65[0:DH].T
                den[b * T:(b + 1) * T, h] = a65[DH]
                i += 1

    # ---- phase 3
    nc3 = _get("p3", build_phase3)
    in3 = [{
        "attn": attn[c * TOK_PC:(c + 1) * TOK_PC],
        "den": den[c * TOK_PC:(c + 1) * TOK_PC],
        "x64": S3 * xf[c * TOK_PC:(c + 1) * TOK_PC],
        "w1h": pack_w1(W1_hi), "w1l": pack_w1(W1_lo), "bf1": bf1_f,
        "w2h": pack_w2(W2_hi), "w2l": pack_w2(W2_lo),
        "bf2": (S3 * np.asarray(b_ff2, f)).reshape(1, D),
        "ones": ones,
        "g2": np.asarray(g2, f).reshape(1, D),
        "b2": np.asarray(b2, f).reshape(1, D),
    } for c in cores]
    r3 = run_bass_kernel_spmd(nc3, in3, cores)
    out = np.concatenate([r3.results[c]["out"] for c in cores], axis=0)
    return out.reshape(B, T, D)


# revision 17
# speedup vs baseline: 1.0466x; 1.0466x over previous
"""Emformer block (pre-LN MHA + FFN, post-LN) on 8 Trainium2 NeuronCores.

Decomposition (zero replicated FLOPs, host reshard between phases):
  Phase 1 (token-sharded, 512 rows/core): LN0 -> qT/kT (channel-major, fp8)
           and v (channel-major, bf16) projections with fp8 DoubleRow
           matmuls (2 contraction tiles per instruction at 0.5 cycles/row).
           LN gamma/beta folded into weights on host; q/k/v scaled by 32 so
           fp8e4m3 sees ~unit-variance data.
  Phase 2 (head-sharded, 4 (b,h) pairs/core): scores^T = k^T.T @ q^T as fp8
           DoubleRow (contraction 64 = 2 k-tiles of 32), exp(s/8192 - 2) on
           ACT straight out of PSUM into bf16, mask multiply on DVE, attn^T
           and the softmax denominator via a 0.5-augmented v column, PSUM
           DMA'd straight to DRAM (no transposes).
  Phase 3 (token-sharded): residual + LN1 -> FFN1/FFN2 in fp8 DoubleRow with
           hi+lo error-split operands (3-term products) so fp8 quantization
           error cancels to ~bf16 level. Whole phase runs at 64x scale and
           relies on layer-norm scale invariance (eps scaled by 64^2) so no
           unscaling ops are needed.
"""

import ml_dtypes
import numpy as np

import concourse.bass as bass
import concourse.mybir as mybir
import concourse.tile as tile
from concourse import bacc
from concourse.bass_utils import run_bass_kernel_spmd
from concourse.masks import make_identity

F32 = mybir.dt.float32
F32R = mybir.dt.float32r
BF16 = mybir.dt.bfloat16
FP8 = mybir.dt.float8e4
AF = mybir.ActivationFunctionType
OP = mybir.AluOpType
DR = mybir.MatmulPerfMode.DoubleRow

NPF8 = ml_dtypes.float8_e4m3
NPBF = ml_dtypes.bfloat16

B, T, D, H, FFN = 2, 2048, 1024, 16, 4096
DH = D // H
LN_EPS = 1e-3
NCORES = 8
NTOK = B * T              # 4096
TOK_PC = NTOK // NCORES   # 512 token rows per core (phases 1/3)
NT = TOK_PC // 128        # 4 token tiles per core
CB = D // 128             # 8 blocks over D
DP = CB // 2              # 4 DoubleRow contraction pairs over D
FB = FFN // 128           # 32 blocks over FFN dim
JP = FB // 2              # 16 DoubleRow contraction pairs over FFN
NPAIR = (B * H) // NCORES # 4 (batch, head) pairs per core (phase 2)
MB = T // 128             # 16 key blocks
NBQ = T // 512            # 4 query blocks of 512

SQ = 32.0                 # q/k/v scale so fp8 sees ~N(0,1)*32 data
SEXP = 0.125 / (SQ * SQ)  # fold 1/sqrt(DH) and the q,k scales into exp
S3 = 64.0                 # phase-3 residual-stream scale (LN invariant)
EPS3 = LN_EPS * S3 * S3


def _ln_stats(nc, pool, xt, eps, rows=128, d=D):
    """Return (negmu, rstd) [rows,1] f32 tiles for layer norm over free dim."""
    nsub = d // 512
    stats = pool.tile([128, nsub, 6], F32, name="ln_stats", tag="ln_stats")
    xg = xt.rearrange("p (s q) -> p s q", s=nsub)
    for s in range(nsub):
        nc.vector.bn_stats(out=stats[:rows, s, :], in_=xg[:rows, s, :])
    mv = pool.tile([128, 2], F32, name="ln_mv", tag="ln_mv")
    nc.vector.bn_aggr(out=mv[:rows], in_=stats[:rows])
    negmu = pool.tile([128, 1], F32, name="ln_negmu", tag="ln_negmu")
    nc.vector.tensor_scalar_mul(negmu[:rows], mv[:rows, 0:1], -1.0)
    std = pool.tile([128, 1], F32, name="ln_std", tag="ln_std")
    eps_t = pool.tile([128, 1], F32, name="ln_eps", tag="ln_eps")
    nc.vector.memset(eps_t, eps)
    nc.scalar.activation(out=std[:rows], in_=mv[:rows, 1:2], func=AF.Sqrt,
                         bias=eps_t[:rows], scale=1.0)
    rstd = pool.tile([128, 1], F32, name="ln_rstd", tag="ln_rstd")
    nc.vector.reciprocal(out=rstd[:rows], in_=std[:rows])
    return negmu, rstd


def build_phase1():
    nc = bacc.Bacc(None, target_bir_lowering=False)
    x_d = nc.dram_tensor("x", [TOK_PC, D], BF16, kind="ExternalInput")
    w_d = {}
    for nm in ("q", "k", "v"):
        for hl in (("h", "l") if nm != "v" else ("h",)):
            w_d[nm, hl] = nc.dram_tensor(f"w{nm}{hl}", [128, DP, 2, D], FP8,
                                         kind="ExternalInput")
    bq_d = nc.dram_tensor("bq", [128, CB], F32, kind="ExternalInput")
    bk_d = nc.dram_tensor("bk", [128, CB], F32, kind="ExternalInput")
    bv_d = nc.dram_tensor("bv", [128, CB], F32, kind="ExternalInput")
    qT_o = nc.dram_tensor("qT", [D, TOK_PC], BF16, kind="ExternalOutput")
    kT_o = nc.dram_tensor("kT", [D, TOK_PC], BF16, kind="ExternalOutput")
    v_o = nc.dram_tensor("v", [D, TOK_PC], BF16, kind="ExternalOutput")

    with tile.TileContext(nc) as tc:
        with (
            tc.tile_pool(name="const", bufs=1) as const,
            tc.tile_pool(name="w", bufs=1) as wpool,
            tc.tile_pool(name="xin", bufs=2) as xin,
            tc.tile_pool(name="small", bufs=4) as small,
            tc.tile_pool(name="ln", bufs=2) as lnp,
            tc.tile_pool(name="lnT", bufs=1) as lnTp,
            tc.tile_pool(name="qout", bufs=4) as qout,
            tc.tile_pool(name="pst", bufs=2, space="PSUM") as pst,
            tc.tile_pool(name="psq", bufs=4, space="PSUM") as psq,
        ):
            xts = []
            for nt in range(NT):
                xt = xin.tile([128, D], BF16, name="xt", tag=f"xt{nt}")
                nc.sync.dma_start(out=xt,
                                  in_=x_d[nt * 128:(nt + 1) * 128, :])
                xts.append(xt)
            ident = const.tile([128, 128], BF16)
            make_identity(nc, ident)
            b_sb = {}
            for name, bd in (("q", bq_d), ("k", bk_d), ("v", bv_d)):
                t = const.tile([128, CB], F32, name=f"b{name}")
                nc.sync.dma_start(out=t, in_=bd[:, :])
                b_sb[name] = t
            w_sb = {}
            for (name, hl), wd in w_d.items():
                t = wpool.tile([128, DP, 2, D], FP8, name=f"w{name}{hl}",
                               tag=f"w{name}{hl}")
                nc.sync.dma_start(out=t, in_=wd[:, :, :, :])
                w_sb[name, hl] = t

            # LN0 -> xn (bf16) -> PE transpose -> lnT fp8 [128, CB, TOK_PC]
            lnT = lnTp.tile([128, CB, TOK_PC], FP8, name="lnT", tag="lnT")
            for nt in range(NT):
                xt = xts[nt]
                negmu, rstd = _ln_stats(nc, small, xt, LN_EPS)
                xn = lnp.tile([128, D], BF16, name="xn", tag="xn")
                nc.vector.tensor_scalar(out=xn, in0=xt, scalar1=negmu,
                                        scalar2=rstd, op0=OP.add, op1=OP.mult)
                for dg in range(2):
                    tp = pst.tile([128, 4, 128], BF16, name="tp", tag="tp")
                    for j in range(4):
                        cb = dg * 4 + j
                        nc.tensor.transpose(
                            tp[:, j, :], xn[:, cb * 128:(cb + 1) * 128],
                            ident)
                    nc.scalar.activation(
                        out=lnT[:, dg * 4:(dg + 1) * 4,
                                nt * 128:(nt + 1) * 128],
                        in_=tp, func=AF.Copy)

            # QKV projections: DoubleRow fp8, out channel-major [128, 512]
            for name, out_d in (("q", qT_o), ("k", kT_o), ("v", v_o)):
                for ob in range(CB):
                    ps = psq.tile([128, TOK_PC], F32, name="psq", tag="psq")
                    hls = ("h", "l") if name != "v" else ("h",)
                    first = True
                    for hl in hls:
                        for dp in range(DP):
                            nc.tensor.matmul(
                                ps,
                                w_sb[name, hl][:, dp, :,
                                               ob * 128:(ob + 1) * 128],
                                lnT[:, 2 * dp:2 * dp + 2, :],
                                start=first,
                                stop=(hl == hls[-1] and dp == DP - 1),
                                perf_mode=DR)
                            first = False
                    ot = qout.tile([128, TOK_PC], BF16, name="qvt",
                                   tag="qkv_out")
                    if name == "v":
                        nc.vector.tensor_scalar(
                            out=ot, in0=ps, scalar1=b_sb[name][:, ob:ob + 1],
                            scalar2=None, op0=OP.add)
                        nc.sync.dma_start(
                            out=out_d[ob * 128:(ob + 1) * 128, :], in_=ot)
                    else:
                        nc.scalar.activation(
                            out=ot, in_=ps, func=AF.Identity,
                            bias=b_sb[name][:, ob:ob + 1], scale=1.0)
                        nc.scalar.dma_start(
                            out=out_d[ob * 128:(ob + 1) * 128, :], in_=ot)

    nc.compile()
    return nc


def build_phase2():
    nc = bacc.Bacc(None, target_bir_lowering=False)
    qT_d = nc.dram_tensor("qT", [NPAIR, DH, T], BF16, kind="ExternalInput")
    kT_d = nc.dram_tensor("kT", [NPAIR, DH, T], BF16, kind="ExternalInput")
    v_d = nc.dram_tensor("v", [NPAIR, T, DH + 1], BF16, kind="ExternalInput")
    mT_d = nc.dram_tensor("maskT", [B, T, T], BF16, kind="ExternalInput")
    attn_o = nc.dram_tensor("attn", [NPAIR, DH + 1, T], F32,
                            kind="ExternalOutput")

    # key-block grouping: (start, size, pool_id) over the 16 key blocks
    GROUPS = [(0, 4, 0), (4, 2, 1), (6, 4, 0), (10, 2, 1), (12, 4, 0)]

    with tile.TileContext(nc) as tc:
        with (
            tc.tile_pool(name="const", bufs=1) as const,
            tc.tile_pool(name="mask", bufs=2) as maskp,
            tc.tile_pool(name="qk", bufs=2) as qkp,
            tc.tile_pool(name="vp", bufs=2) as vp,
            tc.tile_pool(name="em", bufs=3) as emp,
            tc.tile_pool(name="at", bufs=3) as atp,
            tc.tile_pool(name="psA", bufs=1, space="PSUM") as psA,
            tc.tile_pool(name="psB", bufs=1, space="PSUM") as psB,
            tc.tile_pool(name="psa", bufs=2, space="PSUM") as psa,
        ):
            neg2 = const.tile([128, 1], F32)
            nc.vector.memset(neg2, -2.0)
            for b in range(B):
                qs, ks, vs = [], [], []
                for hp in range(2):
                    p = b * 2 + hp
                    qsb = qkp.tile([DH, T], BF16, name=f"qsb{hp}",
                                   tag=f"q{hp}")
                    ksb = qkp.tile([DH, T], BF16, name=f"ksb{hp}",
                                   tag=f"k{hp}")
                    nc.sync.dma_start(out=qsb, in_=qT_d[p])
                    nc.sync.dma_start(out=ksb, in_=kT_d[p])
                    vsb = vp.tile([128, MB, DH + 1], BF16, name=f"vsb{hp}",
                                  tag=f"v{hp}")
                    nc.sync.dma_start(
                        out=vsb,
                        in_=v_d[p].rearrange("(mb mp) d -> mp mb d", mp=128))
                    qs.append(qsb); ks.append(ksb); vs.append(vsb)
                for nb in range(NBQ):
                    ns = slice(nb * 512, (nb + 1) * 512)
                    mt = maskp.tile([128, MB, 512], BF16, name="mt",
                                    tag="mask")
                    nc.sync.dma_start(
                        out=mt,
                        in_=mT_d[b].rearrange("(mb mp) n -> mp mb n",
                                              mp=128)[:, :, ns])
                    for hp in range(2):
                        p = b * 2 + hp
                        qsb, ksb, vsb = qs[hp], ks[hp], vs[hp]
                        em = emp.tile([128, MB, 512], BF16, name="em",
                                      tag="em")
                        for start, size, pid in GROUPS:
                            pool = psA if pid == 0 else psB
                            ps = pool.tile([128, size, 512], F32,
                                           name=f"ps{pid}", tag=f"ps{pid}")
                            for j in range(size):
                                kb = start + j
                                nc.tensor.matmul(
                                    ps[:, j, :],
                                    ksb[:, kb * 128:(kb + 1) * 128],
                                    qsb[:, ns],
                                    start=True, stop=True)
                            sl = slice(start, start + size)
                            nc.scalar.activation(
                                out=em[:, sl, :], in_=ps, func=AF.Exp,
                                scale=SEXP, bias=neg2)
                            nc.vector.tensor_tensor(
                                out=em[:, sl, :], in0=em[:, sl, :],
                                in1=mt[:, sl, :], op=OP.mult)
                        pa = psa.tile([DH + 1, 512], F32, name="pa",
                                      tag="attn")
                        for mb in range(MB):
                            nc.tensor.matmul(pa, vsb[:, mb, :],
                                             em[:, mb, :],
                                             start=(mb == 0),
                                             stop=(mb == MB - 1))
                        aT = atp.tile([DH + 1, 512], F32, name="aT",
                                      tag="aT")
                        nc.vector.tensor_copy(out=aT, in_=pa)
                        nc.gpsimd.dma_start(out=attn_o[p, :, ns], in_=aT)

    nc.compile()
    return nc


def build_phase3():
    nc = bacc.Bacc(None, target_bir_lowering=False)
    attn_d = nc.dram_tensor("attn", [TOK_PC, D], F32, kind="ExternalInput")
    den_d = nc.dram_tensor("den", [TOK_PC, H], F32, kind="ExternalInput")
    x_d = nc.dram_tensor("x64", [TOK_PC, D], F32, kind="ExternalInput")
    w1h_d = nc.dram_tensor("w1h", [FB, 128, DP, 2, 128], FP8,
                           kind="ExternalInput")
    w1l_d = nc.dram_tensor("w1l", [FB, 128, DP, 2, 128], FP8,
                           kind="ExternalInput")
    bf1_d = nc.dram_tensor("bf1", [128, FB], F32, kind="ExternalInput")
    w2h_d = nc.dram_tensor("w2h", [2, 128, JP, 2, 512], FP8,
                           kind="ExternalInput")
    w2l_d = nc.dram_tensor("w2l", [2, 128, JP, 2, 512], FP8,
                           kind="ExternalInput")
    bf2_d = nc.dram_tensor("bf2", [1, D], F32R, kind="ExternalInput")
    ones_d = nc.dram_tensor("ones", [1, 128], F32R, kind="ExternalInput")
    g2_d = nc.dram_tensor("g2", [1, D], F32, kind="ExternalInput")
    b2_d = nc.dram_tensor("b2", [1, D], F32, kind="ExternalInput")
    out_o = nc.dram_tensor("out", [TOK_PC, D], F32, kind="ExternalOutput")

    with tile.TileContext(nc) as tc:
        with (
            tc.tile_pool(name="const", bufs=1) as const,
            tc.tile_pool(name="xin", bufs=2) as xin,
            tc.tile_pool(name="small", bufs=4) as small,
            tc.tile_pool(name="ao", bufs=1) as aop,
            tc.tile_pool(name="ln", bufs=2) as lnp,
            tc.tile_pool(name="lnT", bufs=1) as lnTp,
            tc.tile_pool(name="w1", bufs=2) as w1p,
            tc.tile_pool(name="w2", bufs=1) as w2p,
            tc.tile_pool(name="t1b", bufs=2) as t1bp,
            tc.tile_pool(name="t1", bufs=1) as t1p,
            tc.tile_pool(name="y", bufs=1) as yp,
            tc.tile_pool(name="outp", bufs=2) as outp,
            tc.tile_pool(name="pst", bufs=2, space="PSUM") as pst,
            tc.tile_pool(name="psf", bufs=2, space="PSUM") as psf,
            tc.tile_pool(name="psy", bufs=1, space="PSUM") as psyp,
        ):
            ident = const.tile([128, 128], BF16)
            make_identity(nc, ident)
            ones_t = const.tile([1, 128], F32R)
            nc.sync.dma_start(out=ones_t, in_=ones_d[:, :])
            bf1_sb = const.tile([128, FB], F32)
            nc.sync.dma_start(out=bf1_sb, in_=bf1_d[:, :])
            bf2_sb = const.tile([1, D], F32R)
            nc.sync.dma_start(out=bf2_sb, in_=bf2_d[:, :])
            g2_sb = const.tile([128, D], F32)
            nc.sync.dma_start(out=g2_sb, in_=bass.AP(
                tensor=g2_d, offset=0, ap=[[0, 128], [1, D]]))
            b2_sb = const.tile([128, D], F32)
            nc.sync.dma_start(out=b2_sb, in_=bass.AP(
                tensor=b2_d, offset=0, ap=[[0, 128], [1, D]]))

            # residual 1 (at 64x scale) + LN1 -> transpose -> hi/lo fp8
            lnTh = lnTp.tile([128, CB, TOK_PC], FP8, name="lnTh", tag="lnTh")
            lnTl = lnTp.tile([128, CB, TOK_PC], FP8, name="lnTl", tag="lnTl")
            def load_nt(nt):
                at = xin.tile([128, D], F32, name="at", tag="attn_in")
                nc.sync.dma_start(out=at,
                                  in_=attn_d[nt * 128:(nt + 1) * 128, :])
                xt = xin.tile([128, D], F32, name="xt", tag="x_in")
                nc.sync.dma_start(out=xt,
                                  in_=x_d[nt * 128:(nt + 1) * 128, :])
                den = small.tile([128, H], F32, name="den", tag=f"den{nt}")
                nc.sync.dma_start(out=den,
                                  in_=den_d[nt * 128:(nt + 1) * 128, :])
                return at, xt, den

            ins0 = load_nt(0)
            # FFN2 dh0 weights: no deps, issue early so they never queue
            # behind the dependent w1 stream (dh1 issued mid-FFN1)
            w2_sb = {}
            for nm, wd in (("h", w2h_d), ("l", w2l_d)):
                t = w2p.tile([128, JP, 2, 512], FP8, name=f"w2{nm}0",
                             tag=f"w2{nm}0")
                nc.sync.dma_start(out=t, in_=wd[0])
                w2_sb[nm, 0] = t
            ao_t = []
            for nt in range(NT):
                at, xt, den = ins0 if nt == 0 else load_nt(nt)
                rec = small.tile([128, H], F32, name="recd", tag="recd")
                nc.vector.reciprocal(out=rec, in_=den)
                rec_bc = bass.AP(tensor=rec.tensor, offset=rec.offset,
                                 ap=[rec.ap[0], rec.ap[1], [0, DH]])
                atg = at.rearrange("p (h d) -> p h d", h=H)
                nc.vector.tensor_tensor(out=atg, in0=atg, in1=rec_bc,
                                        op=OP.mult)
                ao = aop.tile([128, D], F32, name=f"ao{nt}", tag=f"ao{nt}")
                nc.vector.tensor_tensor(out=ao, in0=at, in1=xt, op=OP.add)
                ao_t.append(ao)
                negmu, rstd = _ln_stats(nc, small, ao, EPS3)
                ln_t = lnp.tile([128, D], BF16, name="ln1", tag="ln1")
                nc.vector.tensor_scalar(out=ln_t, in0=ao, scalar1=negmu,
                                        scalar2=rstd, op0=OP.add,
                                        op1=OP.mult)
                for dg in range(2):
                    tp = pst.tile([128, 4, 128], BF16, name="tp", tag="tp")
                    for j in range(4):
                        cb = dg * 4 + j
                        nc.tensor.transpose(
                            tp[:, j, :], ln_t[:, cb * 128:(cb + 1) * 128],
                            ident)
                    hsl = lnTh[:, dg * 4:(dg + 1) * 4,
                               nt * 128:(nt + 1) * 128]
                    nc.scalar.activation(out=hsl, in_=tp, func=AF.Copy)
                    nc.vector.tensor_tensor(
                        out=lnTl[:, dg * 4:(dg + 1) * 4,
                                 nt * 128:(nt + 1) * 128],
                        in0=tp, in1=hsl, op=OP.subtract)

            # FFN1 (3-term hi/lo), relu fused on ACT, t1 split hi/lo
            t1h = t1p.tile([128, FB, TOK_PC], FP8, name="t1h", tag="t1h")
            t1l = t1p.tile([128, FB, TOK_PC], FP8, name="t1l", tag="t1l")
            W1CH = 4
            w1ch = {}
            for fb in range(FB):
                if fb % W1CH == 0:
                    for nm, wd in (("h", w1h_d), ("l", w1l_d)):
                        t = w1p.tile([128, W1CH, DP, 2, 128], FP8,
                                     name=f"w1{nm}", tag=f"w1{nm}")
                        nc.sync.dma_start(
                            out=t,
                            in_=wd[fb:fb + W1CH].rearrange(
                                "f p a b m -> p f a b m"))
                        w1ch[nm] = t
                if fb == 8:
                    for nm, wd in (("h", w2h_d), ("l", w2l_d)):
                        t = w2p.tile([128, JP, 2, 512], FP8,
                                     name=f"w2{nm}1", tag=f"w2{nm}1")
                        nc.sync.dma_start(out=t, in_=wd[1])
                        w2_sb[nm, 1] = t
                w1h = w1ch["h"][:, fb % W1CH]
                w1l = w1ch["l"][:, fb % W1CH]
                ps = psf.tile([128, TOK_PC], F32, name="psf", tag="psf")
                first = True
                last = (w1h, lnTl)
                for wt, lt in ((w1h, lnTh), (w1l, lnTh), (w1h, lnTl)):
                    for dp in range(DP):
                        nc.tensor.matmul(
                            ps, wt[:, dp, :, :],
                            lt[:, 2 * dp:2 * dp + 2, :],
                            start=first,
                            stop=(wt is last[0] and lt is last[1]
                                  and dp == DP - 1),
                            perf_mode=DR)
                        first = False
                t1b = t1bp.tile([128, TOK_PC], BF16, name="t1b", tag="t1b")
                nc.scalar.activation(out=t1b, in_=ps, func=AF.Relu,
                                     bias=bf1_sb[:, fb:fb + 1],
                                     scale=1.0 / SQ)
                nc.scalar.activation(out=t1h[:, fb, :], in_=t1b, func=AF.Copy)
                nc.gpsimd.tensor_tensor(out=t1l[:, fb, :], in0=t1b,
                                        in1=t1h[:, fb, :], op=OP.subtract)

            # FFN2 (3-term hi/lo): dh0 jp-outer so it pipelines with the
            # FFN1/t1 stream; dh1 nt-outer so LN2 overlaps the tail.
            y_t = [yp.tile([128, D], F32, name=f"y{nt}", tag=f"y{nt}")
                   for nt in range(NT)]
            dsl0 = slice(0, 512)
            pys = [psyp.tile([128, 512], F32, name=f"psy{nt}", tag=f"psy{nt}")
                   for nt in range(NT)]
            for jp in range(JP):
                for tt, wn in ((t1h, "h"), (t1h, "l"), (t1l, "h")):
                    w2 = w2_sb[wn, 0]
                    for nt in range(NT):
                        nc.tensor.matmul(
                            pys[nt],
                            tt[:, 2 * jp:2 * jp + 2,
                               nt * 128:(nt + 1) * 128],
                            w2[:, jp, :, :],
                            start=(jp == 0 and wn == "h" and tt is t1h),
                            stop=False, perf_mode=DR)
            for nt in range(NT):
                nc.tensor.matmul(pys[nt], ones_t, bf2_sb[0:1, dsl0],
                                 start=False, stop=True)
                nc.vector.tensor_tensor(out=y_t[nt][:, dsl0], in0=pys[nt],
                                        in1=ao_t[nt][:, dsl0], op=OP.add)
            dsl1 = slice(512, 1024)
            for nt in range(NT):
                tsl = slice(nt * 128, (nt + 1) * 128)
                py = psyp.tile([128, 512], F32, name=f"psyb{nt}",
                               tag=f"psy{nt}")
                first = True
                for tt, wn in ((t1h, "h"), (t1h, "l"), (t1l, "h")):
                    w2 = w2_sb[wn, 1]
                    for jp in range(JP):
                        nc.tensor.matmul(
                            py, tt[:, 2 * jp:2 * jp + 2, tsl],
                            w2[:, jp, :, :],
                            start=first, stop=False, perf_mode=DR)
                        first = False
                nc.tensor.matmul(py, ones_t, bf2_sb[0:1, dsl1],
                                 start=False, stop=True)
                nc.vector.tensor_tensor(out=y_t[nt][:, dsl1], in0=py,
                                        in1=ao_t[nt][:, dsl1], op=OP.add)
                negmu, rstd = _ln_stats(nc, small, y_t[nt], EPS3)
                z = lnp.tile([128, D], F32, name="z", tag="z")
                nc.vector.tensor_scalar(out=z, in0=y_t[nt], scalar1=negmu,
                                        scalar2=rstd, op0=OP.add,
                                        op1=OP.mult)
                ot = outp.tile([128, D], F32, name="ot", tag="out")
                nc.vector.tensor_tensor(out=ot, in0=z, in1=g2_sb, op=OP.mult)
                nc.vector.tensor_tensor(out=ot, in0=ot, in1=b2_sb, op=OP.add)
                nc.sync.dma_start(out=out_o[tsl, :], in_=ot)

    nc.compile()
    return nc


_CACHE = {}


def _get(name, builder):
    if name not in _CACHE:
        _CACHE[name] = builder()
    return _CACHE[name]


def _pack_w_in(w):
    """[D, N] -> DoubleRow lhsT layout [128, DP, 2, N] (fp8)."""
    return np.ascontiguousarray(
        np.asarray(w, np.float32).reshape(DP, 2, 128, -1)
        .transpose(2, 0, 1, 3)).astype(NPF8)


def kernel(x, mask, Wq, bq, Wk, bk, Wv, bv, g_in, b_in, g1, b1,
           W_ff1, b_ff1, W_ff2, b_ff2, g2, b2):
    f = np.float32
    x = np.asarray(x, f)
    xf = x.reshape(NTOK, D)
    Wq_s = SQ * (np.asarray(g_in, f)[:, None] * np.asarray(Wq, f))
    Wk_s = SQ * (np.asarray(g_in, f)[:, None] * np.asarray(Wk, f))
    Wv_s = SQ * (np.asarray(g_in, f)[:, None] * np.asarray(Wv, f))
    bq_f = (SQ * (b_in @ Wq + bq)).astype(f).reshape(CB, 128).T.copy()
    bk_f = (SQ * (b_in @ Wk + bk)).astype(f).reshape(CB, 128).T.copy()
    bv_f = (SQ * (b_in @ Wv + bv)).astype(f).reshape(CB, 128).T.copy()
    maskT = np.ascontiguousarray(
        np.asarray(mask)[:, 0].transpose(0, 2, 1)).astype(NPBF)

    W1_s = SQ * (np.asarray(g1, f)[:, None] * np.asarray(W_ff1, f))
    W1_hi = W1_s.astype(NPF8)
    W1_lo = (W1_s - W1_hi.astype(f)).astype(NPF8)
    bf1_f = (b1 @ W_ff1 + b_ff1).astype(f).reshape(FB, 128).T.copy()
    W2_s = S3 * np.asarray(W_ff2, f)
    W2_hi = W2_s.astype(NPF8)
    W2_lo = (W2_s - W2_hi.astype(f)).astype(NPF8)

    def pack_w1(w8):
        # [D, FFN] fp8 -> [FB, 128, DP, 2, 128]
        return np.ascontiguousarray(
            w8.reshape(DP, 2, 128, FB, 128).transpose(3, 2, 0, 1, 4))

    def pack_w2(w8):
        # [FFN, D] fp8 -> [2, 128, JP, 2, 512]
        return np.ascontiguousarray(
            w8.reshape(JP, 2, 128, 2, 512).transpose(3, 2, 0, 1, 4))

    ones = np.ones((1, 128), f)
    cores = list(range(NCORES))

    # ---- phase 1
    nc1 = _get("p1", build_phase1)
    w_in = {}
    for nm, Ws in (("q", Wq_s), ("k", Wk_s), ("v", Wv_s)):
        hi = Ws.astype(NPF8)
        lo = (Ws - hi.astype(f)).astype(NPF8)
        w_in[f"w{nm}h"] = _pack_w_in(hi)
        w_in[f"w{nm}l"] = _pack_w_in(lo)
    in1 = [{
        "x": xf[c * TOK_PC:(c + 1) * TOK_PC].astype(NPBF),
        "bq": bq_f, "bk": bk_f, "bv": bv_f, **w_in,
    } for c in cores]
    r1 = run_bass_kernel_spmd(nc1, in1, cores)
    qT = np.concatenate([r1.results[c]["qT"] for c in cores], axis=1)
    kT = np.concatenate([r1.results[c]["kT"] for c in cores], axis=1)
    vT = np.concatenate([r1.results[c]["v"] for c in cores], axis=1)

    # ---- phase 2
    nc2 = _get("p2", build_phase2)
    in2 = []
    for c in cores:
        qs, ks, vs = [], [], []
        for b in range(B):
            for hp in range(2):
                h = 2 * c + hp
                dsl = slice(h * DH, (h + 1) * DH)
                tsl = slice(b * T, (b + 1) * T)
                qs.append(qT[dsl, tsl])
                ks.append(kT[dsl, tsl])
                va = np.empty((T, DH + 1), NPBF)
                va[:, 0:DH] = vT[dsl, tsl].T
                va[:, DH] = NPBF(0.5)
                vs.append(va)
        in2.append({
            "qT": np.ascontiguousarray(np.stack(qs)),
            "kT": np.ascontiguousarray(np.stack(ks)),
            "v": np.ascontiguousarray(np.stack(vs)),
            "maskT": maskT,
        })
    r2 = run_bass_kernel_spmd(nc2, in2, cores)
    attn = np.empty((NTOK, D), f)
    den = np.empty((NTOK, H), f)
    for c in cores:
        i = 0
        for b in range(B):
            for hp in range(2):
                h = 2 * c + hp
                a65 = r2.results[c]["attn"][i]
                attn[b * T:(b + 1) * T, h * DH:(h + 1) * DH] = a65[0:DH].T
                den[b * T:(b + 1) * T, h] = a65[DH]
                i += 1

    # ---- phase 3
    nc3 = _get("p3", build_phase3)
    in3 = [{
        "attn": attn[c * TOK_PC:(c + 1) * TOK_PC],
        "den": den[c * TOK_PC:(c + 1) * TOK_PC],
        "x64": S3 * xf[c * TOK_PC:(c + 1) * TOK_PC],
        "w1h": pack_w1(W1_hi), "w1l": pack_w1(W1_lo), "bf1": bf1_f,
        "w2h": pack_w2(W2_hi), "w2l": pack_w2(W2_lo),
        "bf2": (S3 * np.asarray(b_ff2, f)).reshape(1, D),
        "ones": ones,
        "g2": np.asarray(g2, f).reshape(1, D),
        "b2": np.asarray(b2, f).reshape(1, D),
    } for c in cores]
    r3 = run_bass_kernel_spmd(nc3, in3, cores)
    out = np.concatenate([r3.results[c]["out"] for c in cores], axis=0)
    return out.reshape(B, T, D)
